# Initial kernel scaffold
#
"""Trainium2 Bass kernel for nn_Conv2d_res_v3 (Sobel-gradient gabor CNN).

Computes, for x [128,3,224,224] f32 and mask [3,224,224] f32:
    out_normal = x * mask
    gx = dwconv3x3(x, KX), gy = dwconv3x3(x, KY)   (KX/KY separable Sobel-like)
    gabor = atan(sqrt((gx/(x+1e-3))^2 + (gy/(x+1e-3))^2))
    out = concat([out_normal, gabor], axis=1);  returns (out, x)

Strategy: pure data-parallel over batch across 8 NeuronCores (16 imgs/core).
Per core: 96 tiles of [112 rows, 224 cols] (plane halves, 1 halo row).
PE does both convs as banded-matmul column-shifted PSUM accumulations
(both kernels are separable; the vertical factor becomes a [113,112]
banded lhsT, the horizontal factor becomes +/-1-column-shifted matmuls).
Elementwise chain is phased so the ACT engine needs only two table sets:
  phase A: square / abs_reciprocal_sqrt, phase B: arctan, via the identity
  atan(u) = pi/4 + atan(1 - 2/(1+u)),  u = sqrt(s)/|d|.
"""

import sys

sys.path.insert(0, "/opt/trn_rl_repo")

import numpy as np

import concourse.bass as bass
import concourse.tile as tile
from concourse import mybir
from concourse.bass_utils import run_bass_kernel_spmd

N_CORES = 8
NB = 16          # images per core
C = 3
H = W = 224
HALF = 112       # rows per tile
KR = 113         # rows loaded per tile (HALF + 1 halo)
A = 1.0 / (2.0 * np.sqrt(2.0))
F32 = mybir.dt.float32
AF = mybir.ActivationFunctionType
OP = mybir.AluOpType

_CACHE = {}


def _vmats():
    """Banded [KR,112] lhsT matrices for vertical blur [a,1,a] and diff [1,0,-1].

    Top half: out rows 0..111 of the plane, input rows 0..112 (row -1 zero-pad).
      lhsT[k,m] = taps[k-m+1]  (k: input row, m: output row)
    Bottom half: out rows 112..223, input rows 111..223 (row 224 zero-pad).
      lhsT[k,m] = taps[k-m]    (input row 111+k, output row 112+m)
    """
    def band(taps, off):
        M = np.zeros((KR, HALF), np.float32)
        for m in range(HALF):
            for i, t in enumerate(taps):
                k = m + i - 1 + off
                if 0 <= k < KR:
                    M[k, m] = t
        return M

    blur = [A, 1.0, A]
    diff = [1.0, 0.0, -1.0]
    mats = []
    for off in (0, 1):  # top, bottom
        Bm = band(blur, off)
        Dm = band(diff, off)
        mats += [Bm, -Bm, Dm, A * Dm]
    return np.ascontiguousarray(np.stack(mats))  # [8, KR, 112]


WTS = _vmats()


def _build():
    from concourse.dve_ops import RECIPROCAL_APPROX_FAST

    nc = bass.Bass()
    x_d = nc.declare_dram_parameter("x", [NB, C, H, W], F32, isOutput=False)
    mask_d = nc.declare_dram_parameter("mask", [C, H, W], F32, isOutput=False)
    wts_d = nc.declare_dram_parameter("wts", list(WTS.shape), F32, isOutput=False)
    on_d = nc.declare_dram_parameter("on", [NB, C, H, W], F32, isOutput=True)
    gb_d = nc.declare_dram_parameter("gb", [NB, C, H, W], F32, isOutput=True)

    ntiles = NB * C * 2

    with tile.TileContext(nc) as tc:
        with (
            tc.tile_pool(name="const", bufs=1) as const,
            tc.tile_pool(name="xin", bufs=4) as xin,
            tc.tile_pool(name="gxp", bufs=2, space="PSUM") as gxp,
            tc.tile_pool(name="gyp", bufs=2, space="PSUM") as gyp,
            tc.tile_pool(name="ew", bufs=3) as ew,
            tc.tile_pool(name="zzs", bufs=ntiles) as zzs,
            tc.tile_pool(name="outb", bufs=4) as outb,
        ):
            # constants: 8 weight matrices, 6 mask half-planes, ones
            wt_sb = const.tile([KR, 8 * HALF], F32)
            for i in range(8):
                nc.sync.dma_start(
                    out=wt_sb[:, i * HALF:(i + 1) * HALF], in_=wts_d[i]
                )
            mask_sb = const.tile([HALF, 6 * W], F32)
            for c in range(C):
                for h in range(2):
                    j = c * 2 + h
                    nc.sync.dma_start(
                        out=mask_sb[:, j * W:(j + 1) * W],
                        in_=mask_d[c, h * HALF:(h + 1) * HALF, :],
                    )
            ones_t = const.tile([HALF, W], F32)
            nc.vector.memset(ones_t, 1.0)

            def wt(half, kind):
                i = half * 4 + {"B": 0, "Bn": 1, "D": 2, "aD": 3}[kind]
                return wt_sb[:, i * HALF:(i + 1) * HALF]

            zz_tiles = []

            # ---------------- phase A ----------------
            for t in range(ntiles):
                n, rem = divmod(t, C * 2)
                c, half = divmod(rem, 2)
                r0 = 0 if half == 0 else H - KR  # 0 or 111

                x_t = xin.tile([KR, W], F32)
                nc.sync.dma_start(out=x_t, in_=x_d[n, c, r0:r0 + KR, :])
                xc = x_t[0:HALF] if half == 0 else x_t[1:1 + HALF]

                gx = gxp.tile([HALF, W], F32)
                gy = gyp.tile([HALF, W], F32)
                nc.tensor.matmul(
                    gx[:, 0:W - 1], wt(half, "B"), x_t[:, 1:W],
                    start=True, stop=False,
                )
                nc.tensor.matmul(
                    gx[:, 1:W], wt(half, "Bn"), x_t[:, 0:W - 1],
                    start=False, stop=True,
                )
                nc.tensor.matmul(
                    gy, wt(half, "D"), x_t, start=True, stop=False
                )
                nc.tensor.matmul(
                    gy[:, 0:W - 1], wt(half, "aD"), x_t[:, 1:W],
                    start=False, stop=False,
                )
                nc.tensor.matmul(
                    gy[:, 1:W], wt(half, "aD"), x_t[:, 0:W - 1],
                    start=False, stop=True,
                )

                gx2 = ew.tile([HALF, W], F32, tag="gx2")
                nc.scalar.activation(gx2, gx, AF.Square)
                gy2 = ew.tile([HALF, W], F32, tag="gy2")
                nc.scalar.activation(gy2, gy, AF.Square)
                s = ew.tile([HALF, W], F32, tag="s")
                nc.vector.scalar_tensor_tensor(
                    s, gx2, 1e-30, gy2, op0=OP.max, op1=OP.add
                )
                irs = ew.tile([HALF, W], F32, tag="irs")
                nc.scalar.activation(irs, s, AF.Abs_reciprocal_sqrt)
                qs = ew.tile([HALF, W], F32, tag="qs")
                nc.vector.scalar_tensor_tensor(
                    qs, xc, 0.001, irs, op0=OP.add, op1=OP.mult
                )
                den = ew.tile([HALF, W], F32, tag="den")
                nc.vector.scalar_tensor_tensor(
                    den, qs, 0.0, ones_t, op0=OP.abs_max, op1=OP.add
                )
                zz = zzs.tile([HALF, W], F32)
                nc.vector.reciprocal_approx_fast(out=zz, in_=den)
                zz_tiles.append(zz)

                on_t = outb.tile([HALF, W], F32, tag="on")
                nc.gpsimd.tensor_mul(
                    on_t, xc, mask_sb[:, (c * 2 + half) * W:(c * 2 + half + 1) * W]
                )
                nc.sync.dma_start(
                    out=on_d[n, c, half * HALF:(half + 1) * HALF, :], in_=on_t
                )

            # ---------------- phase B ----------------
            for t in range(ntiles):
                n, rem = divmod(t, C * 2)
                c, half = divmod(rem, 2)
                g_t = outb.tile([HALF, W], F32, tag="g")
                # atan(2*zz-1) in [-pi/4, pi/4]; +pi/4 handled below
                nc.scalar.activation(
                    g_t, zz_tiles[t], AF.Arctan, bias=-1.0, scale=2.0
                )
                nc.vector.tensor_scalar_add(g_t, g_t, np.float32(np.pi / 4))
                nc.sync.dma_start(
                    out=gb_d[n, c, half * HALF:(half + 1) * HALF, :], in_=g_t
                )

    return nc


def _get_nc():
    if "nc" not in _CACHE:
        _CACHE["nc"] = _build()
    return _CACHE["nc"]


def kernel(x, mask):
    x = np.ascontiguousarray(x, dtype=np.float32)
    mask = np.ascontiguousarray(mask, dtype=np.float32)
    nc = _get_nc()
    in_maps = [
        {"x": x[i * NB:(i + 1) * NB], "mask": mask, "wts": WTS}
        for i in range(N_CORES)
    ]
    res = run_bass_kernel_spmd(nc, in_maps, list(range(N_CORES))).results
    out = np.empty((N_CORES * NB, 2 * C, H, W), np.float32)
    for i in range(N_CORES):
        out[i * NB:(i + 1) * NB, 0:C] = res[i]["on"]
        out[i * NB:(i + 1) * NB, C:2 * C] = res[i]["gb"]
    return (out, x)


# revision 21
# speedup vs baseline: 1.2197x; 1.2197x over previous
"""Trainium2 Bass kernel for nn_Conv2d_res_v3 (Sobel-gradient gabor CNN).

For x [128,3,224,224] f32 and mask [3,224,224] f32:
    out_normal = x * mask
    gx = dwconv3x3(x, KX), gy = dwconv3x3(x, KY)   (separable Sobel-like)
    gabor = atan(sqrt((gx/(x+1e-3))^2 + (gy/(x+1e-3))^2))
    out = concat([out_normal, gabor], axis=1);  returns (out, x)

Pure data-parallel over batch across 8 NeuronCores (16 imgs/core).
Per core, the 48 image planes are processed as 24 PAIRS of consecutive
planes packed side-by-side in the free dim: tiles are [112 rows, 448]
(two 224-wide planes), which halves both the DMA instruction count
(the descriptor-generation serial cost dominates DMA time) and the
per-instruction overhead of every compute op. Each pair yields a top
and a bottom "dtile" (plane halves + 1 halo row -> x tiles [113,448]).

PE computes both convs as banded-matmul column-shifted PSUM
accumulations (the vertical conv factor is a [113,112] banded lhsT;
the horizontal factor becomes +/-1-column-shifted matmuls, split per
plane block so shifts never cross the pair seam).

Elementwise chain (fused custom DVE ops + ACT), phased per group so the
ACT engine's table sets switch only NGROUPS*2-1 times:
    s   = gx^2 + gy^2            [ACT Square + custom DVE ADD_SQ]
    r   = sqrt(s + 1e-38)        [ACT Sqrt]
    den = |x + 1e-3| + r         [custom DVE ABS_ADD]
    zz  = r * recip(den)         [DVE reciprocal_approx_fast + STT]
    gabor = pi/4 + atan(2*zz-1)  [ACT Arctan affine + DVE TS]
using atan(u) = pi/4 + atan((u-1)/(u+1)), u = r/|d|.
"""

import sys

sys.path.insert(0, "/opt/trn_rl_repo")

import numpy as np

import concourse.bacc as bacc
import concourse.bass as bass
import concourse.tile as tile
from concourse import mybir
from concourse.bass_utils import run_bass_kernel_spmd
from concourse.tile_rust import add_dep_helper

N_CORES = 8
NB = 16          # images per core
C = 3
H = W = 224
HALF = 112       # rows per dtile
KR = 113         # rows loaded per dtile (HALF + 1 halo)
W2 = 2 * W       # packed free dim
NGROUPS = 2
A = 1.0 / (2.0 * np.sqrt(2.0))
F32 = mybir.dt.float32
AF = mybir.ActivationFunctionType
OP = mybir.AluOpType

_CACHE = {}

# ---------------- custom DVE ops (registered at import) ----------------


def _register_custom_ops():
    import concourse.dve_ops as dom
    from concourse.dve_spec import (
        Spec, Src0, Src1, C0, Zero, maxx, sq, lower as dve_lower,
        _has_src1,
    )
    from concourse.dve_uop import DveOpSpec

    def reg(name, spec):
        for o in dom.OPS:
            if o.name == name:
                return o
        row = max(dom._SUB_OPCODE_FOR_NAME.values()) + 1
        assert row < 0x20
        dom._SUB_OPCODE_FOR_NAME[name] = row
        shas = {}
        for ver in ("v3", "v4"):
            try:
                uops = dve_lower(spec, ver=ver)
            except Exception:
                continue
            shas[ver] = DveOpSpec(
                name=name, opcode=row, uops=uops, rd1_en=_has_src1(spec)
            ).sha(ver)
        op = dom.DveOp(name, spec, subdim=False, uops_sha=shas)
        dom.OPS.append(op)
        dom.CUSTOM_DVE_SPECS[name] = spec
        return op

    # s = in0 + in1^2
    add_sq = reg(
        "ADD_SQ_ANT",
        Spec(
            body=Src0 + sq(Src1),
            reference=lambda in0, in1, c0, c1, c2: in0 + in1 * in1,
        ),
    )
    # den = |in0 + c0| + in1
    _t = Src0 + C0
    abs_add = reg(
        "ABS_ADD_ANT",
        Spec(
            body=maxx(_t, Zero - _t) + Src1,
            reference=lambda in0, in1, c0, c1, c2: np.abs(in0 + c0) + in1,
        ),
    )
    return add_sq, abs_add


ADD_SQ_OP, ABS_ADD_OP = _register_custom_ops()


def _vmats():
    """Banded [KR,112] lhsT matrices for vertical blur [a,1,a] / diff [1,0,-1].

    x-tile partition layouts (output rows at partition base 0):
      top half:    p -> row p       (rows 0..112; halo row 112 at p=112)
      bottom half: p in [0,111] -> row 112+p; p=112 -> halo row 111
    lhsT[k,m] = weight of input partition k for output partition m.
    """
    def band(taps, half):
        M = np.zeros((KR, HALF), np.float32)
        for m in range(HALF):
            R = m if half == 0 else 112 + m
            for i, t in enumerate(taps):
                r = R + i - 1
                if t == 0.0 or r < 0 or r > H - 1:
                    continue
                if half == 0:
                    k = r
                else:
                    k = 112 if r == 111 else r - 112
                M[k, m] = t
        return M

    blur = [A, 1.0, A]
    diff = [1.0, 0.0, -1.0]
    mats = []
    for half in (0, 1):
        Bm = band(blur, half)
        Dm = band(diff, half)
        mats += [Bm, -Bm, Dm, A * Dm]
    return np.ascontiguousarray(np.stack(mats))  # [8, KR, 112]


WTS = _vmats()


def prep_mask(mask):
    """[3,224,224] -> [6,112,448]: mask tiles for (pair%3, half) patterns.

    Pair p covers planes (2p, 2p+1) with channels ((2p)%3, (2p+1)%3),
    a pattern that repeats every 3 pairs.
    """
    mp = np.empty((6, HALF, W2), np.float32)
    for pp in range(3):
        c0, c1 = (2 * pp) % 3, (2 * pp + 1) % 3
        for half in range(2):
            rows = slice(half * HALF, (half + 1) * HALF)
            mp[pp * 2 + half, :, :W] = mask[c0, rows]
            mp[pp * 2 + half, :, W:] = mask[c1, rows]
    return np.ascontiguousarray(mp)


def _pair_ap(base_ap, plane0, row0, nrows):
    """DRAM AP covering rows [row0, row0+nrows) of planes plane0, plane0+1
    as a [nrows, 2, W] pattern (partition, plane-block, col)."""
    return bass.AP(
        tensor=base_ap.tensor,
        offset=base_ap.offset + plane0 * H * W + row0 * W,
        ap=[[W, nrows], [H * W, 2], [1, W]],
    )


def _build(reps=1):
    nc = bacc.Bacc()
    for v in (2.0, -1.0, 1e-38):
        t = nc.alloc_sbuf_tensor(f"const-f32-{v}", [128, 1], F32)
        nc.gpsimd.memset(t.ap(), v)
        nc.const_aps.aps[(F32, v)] = t.ap()
    nc.all_engine_barrier()

    x_d = nc.declare_dram_parameter("x", [NB, C, H, W], F32, isOutput=False)
    mask_d = nc.declare_dram_parameter("maskp", [6, HALF, W2], F32,
                                       isOutput=False)
    wts_d = nc.declare_dram_parameter("wts", list(WTS.shape), F32,
                                      isOutput=False)
    on_d = nc.declare_dram_parameter("on", [NB, C, H, W], F32, isOutput=True)
    gb_d = nc.declare_dram_parameter("gb", [NB, C, H, W], F32, isOutput=True)
    x_ap = x_d[:, :, :, :]
    on_ap = on_d[:, :, :, :]
    gb_ap = gb_d[:, :, :, :]

    NDT = NPAIRS * 2 if False else (NB * C // 2) * 2  # dtiles
    npairs = NB * C // 2

    with tile.TileContext(nc) as tc:
        with (
            tc.tile_pool(name="const", bufs=1) as const,
            tc.tile_pool(name="xin", bufs=8) as xin,
            tc.tile_pool(name="gxp", bufs=3, space="PSUM") as gxp,
            tc.tile_pool(name="gyp", bufs=3, space="PSUM") as gyp,
            tc.tile_pool(name="ew", bufs=6) as ew,
            tc.tile_pool(name="zzs", bufs=NDT // NGROUPS + 2) as zzs,
            tc.tile_pool(name="outb", bufs=8) as outb,
        ):
            wt_sb = const.tile([KR, 8 * HALF], F32)
            nc.sync.dma_start(
                out=wt_sb[:].rearrange("k (i m) -> k i m", i=8),
                in_=wts_d[:, :, :].rearrange("i k m -> k i m"),
            )
            mask_sb = const.tile([HALF, 6 * W2], F32)
            nc.sync.dma_start(
                out=mask_sb[:].rearrange("k (j m) -> k j m", j=6),
                in_=mask_d[:, :, :].rearrange("j k m -> k j m"),
            )

            def wt(half, kind):
                i = half * 4 + {"B": 0, "Bn": 1, "D": 2, "aD": 3}[kind]
                return wt_sb[:, i * HALF:(i + 1) * HALF]

            import contextlib
            loop_cm = tc.For_i(0, reps, 1) if reps > 1 else (
                contextlib.nullcontext())
            with loop_cm:
                last_arctan = None
                for g in range(NGROUPS):
                    dts = list(range(g * NDT // NGROUPS,
                                     (g + 1) * NDT // NGROUPS))
                    zz_tiles = {}
                    last_act_a = None

                    # ---------------- phase A ----------------
                    for t in dts:
                        p, half = divmod(t, 2)
                        j0 = 2 * p

                        x_t = xin.tile([KR, W2], F32)
                        xv = x_t[:].rearrange("k (b m) -> k b m", b=2)
                        if half == 0:
                            nc.sync.dma_start(
                                out=xv, in_=_pair_ap(x_ap, j0, 0, KR)
                            )
                        else:
                            nc.sync.dma_start(
                                out=xv[0:HALF],
                                in_=_pair_ap(x_ap, j0, HALF, HALF),
                            )
                            nc.sync.dma_start(
                                out=xv[HALF:KR],
                                in_=_pair_ap(x_ap, j0, HALF - 1, 1),
                            )
                        xc = x_t[0:HALF]

                        gx = gxp.tile([HALF, W2], F32)
                        gy = gyp.tile([HALF, W2], F32)
                        # gx = Bv (x) Dh : per plane block b, shifted matmuls
                        for b in (0, 1):
                            o = b * W
                            nc.tensor.matmul(
                                gx[:, o:o + W - 1], wt(half, "B"),
                                x_t[:, o + 1:o + W],
                                start=(b == 0), stop=False,
                            )
                        for b in (0, 1):
                            o = b * W
                            nc.tensor.matmul(
                                gx[:, o + 1:o + W - 1], wt(half, "Bn"),
                                x_t[:, o:o + W - 2],
                                start=False, stop=False,
                            )
                            # last col of block: has_written still clear ->
                            # the accumulate becomes an overwrite
                            nc.tensor.matmul(
                                gx[:, o + W - 1:o + W], wt(half, "Bn"),
                                x_t[:, o + W - 2:o + W - 1],
                                start=False, stop=(b == 1),
                            )
                        # gy = Dv (x) Bh : center spans the seam (no shift)
                        nc.tensor.matmul(
                            gy, wt(half, "D"), x_t, start=True, stop=False
                        )
                        for b in (0, 1):
                            o = b * W
                            nc.tensor.matmul(
                                gy[:, o:o + W - 1], wt(half, "aD"),
                                x_t[:, o + 1:o + W],
                                start=False, stop=False,
                            )
                            nc.tensor.matmul(
                                gy[:, o + 1:o + W], wt(half, "aD"),
                                x_t[:, o:o + W - 1],
                                start=False, stop=(b == 1),
                            )

                        gx2 = ew.tile([HALF, W2], F32, tag="gx2")
                        a1 = nc.scalar.activation(gx2, gx, AF.Square)
                        s = ew.tile([HALF, W2], F32, tag="s")
                        nc.vector._custom_dve(ADD_SQ_OP, out=s, in0=gx2,
                                              in1=gy)
                        r = ew.tile([HALF, W2], F32, tag="r")
                        # bias keeps r >= 1e-19 so den > 0 for recip approx
                        a2 = nc.scalar.activation(r, s, AF.Sqrt, bias=1e-38)
                        if last_arctan is not None:
                            add_dep_helper(a1.ins, last_arctan.ins,
                                           sync=False, reason="act grp")
                            add_dep_helper(a2.ins, last_arctan.ins,
                                           sync=False, reason="act grp")
                        last_act_a = a2
                        den = ew.tile([HALF, W2], F32, tag="den")
                        nc.vector._custom_dve(ABS_ADD_OP, out=den, in0=xc,
                                              in1=r, s0=0.001)
                        iden = ew.tile([HALF, W2], F32, tag="iden")
                        nc.vector.reciprocal_approx_fast(out=iden, in_=den)
                        # zz = r/(r+|d|) in [0,1]; arctan arg 2zz-1 in [-1,1]
                        zz = zzs.tile([HALF, W2], F32)
                        nc.vector.scalar_tensor_tensor(
                            zz, r, 0.0, iden, op0=OP.bypass, op1=OP.mult
                        )
                        zz_tiles[t] = zz

                        on_t = outb.tile([HALF, W2], F32, tag="on")
                        q = (p % 3) * 2 + half
                        nc.gpsimd.tensor_mul(
                            on_t, xc, mask_sb[:, q * W2:(q + 1) * W2]
                        )
                        nc.sync.dma_start(
                            out=_pair_ap(on_ap, j0, half * HALF, HALF),
                            in_=on_t[:].rearrange("k (b m) -> k b m", b=2),
                        )

                    # ---------------- phase B ----------------
                    for t in dts:
                        p, half = divmod(t, 2)
                        j0 = 2 * p
                        g_t = outb.tile([HALF, W2], F32, tag="g")
                        arct = nc.scalar.activation(
                            g_t, zz_tiles[t], AF.Arctan, bias=-1.0, scale=2.0
                        )
                        add_dep_helper(arct.ins, last_act_a.ins, sync=False,
                                       reason="act grp")
                        last_arctan = arct
                        nc.vector.tensor_scalar_add(
                            g_t, g_t, float(np.pi / 4)
                        )
                        nc.sync.dma_start(
                            out=_pair_ap(gb_ap, j0, half * HALF, HALF),
                            in_=g_t[:].rearrange("k (b m) -> k b m", b=2),
                        )

    nc.compile()
    return nc


def _get_nc():
    if "nc" not in _CACHE:
        _CACHE["nc"] = _build()
    return _CACHE["nc"]


def kernel(x, mask):
    x = np.ascontiguousarray(x, dtype=np.float32)
    mask = np.ascontiguousarray(mask, dtype=np.float32)
    maskp = prep_mask(mask)
    nc = _get_nc()
    in_maps = [
        {"x": x[i * NB:(i + 1) * NB], "maskp": maskp, "wts": WTS}
        for i in range(N_CORES)
    ]
    res = run_bass_kernel_spmd(nc, in_maps, list(range(N_CORES))).results
    out = np.empty((N_CORES * NB, 2 * C, H, W), np.float32)
    for i in range(N_CORES):
        out[i * NB:(i + 1) * NB, 0:C] = res[i]["on"]
        out[i * NB:(i + 1) * NB, C:2 * C] = res[i]["gb"]
    return (out, x)
